# revision 16
# baseline (speedup 1.0000x reference)
"""BitLinear + tanh + weighted cumsum + phase-wrap head, 8-way batch-parallel
on one TRN2 chip (8 NeuronCores).

Math (per batch element, matching the BitNet b1.58 reference forward pass):
  amax_t  = max(max_d |x[t,d]|, 1e-5)
  xi[t,d] = rne(x[t,d] * 127/amax_t)            # ints in [-127,127]
  mw      = max(mean|W|, 1e-5)
  wi[o,d] = clip(rne(W[o,d]/mw), -1, 1)         # ternary ints
  I[o,t]  = sum_d xi[t,d]*wi[o,d]               # EXACT in bf16 matmul + f32 PSUM
  inst    = tanh((I * amax_t/127) * mw + b[o])
  S[o,t]  = cumsum_t inst
  out     = c*S - 2pi*rne(c*S/2pi),  c = pi*cumsum_weight

All rounding uses the fp32 magic constant 1.5*2^23 (round-to-nearest-even).
"""

import os
import sys

for _p in ("/opt/trn_rl_repo", "/root/.axon_site/_ro/trn_rl_repo"):
    if os.path.isdir(_p) and _p not in sys.path:
        sys.path.insert(0, _p)

import numpy as np
from contextlib import ExitStack

import concourse.bass as bass
from concourse import bacc
from concourse import mybir
from concourse.bass_utils import run_bass_kernel_spmd
from concourse.tile import TileContext
from concourse.masks import make_identity

F32 = mybir.dt.float32
BF16 = mybir.dt.bfloat16
MAGIC = 12582912.0  # 1.5 * 2**23, fp32 round-to-nearest-even trick
PI = float(np.pi)
TWO_PI = 2.0 * PI
N_CORES = 8
Alu = mybir.AluOpType
Act = mybir.ActivationFunctionType


def build(cw: float, T: int = 4096, D: int = 1024, O: int = 1024):
    """Build the per-core Bass program. Every core runs the same NEFF on its
    own batch shard."""
    NTT = T // 128      # number of 128-row t-tiles
    NCH = T // 512      # number of 512-col t-chunks
    NK = D // 128       # contraction sub-tiles
    NO = O // 128       # output o-tiles
    c_coef = PI * cw
    g_coef = c_coef / TWO_PI

    nc = bacc.Bacc("TRN2", target_bir_lowering=False, debug=False)
    x_d = nc.dram_tensor("x", [T, D], F32, kind="ExternalInput")
    w_d = nc.dram_tensor("W", [O, D], F32, kind="ExternalInput")
    b_d = nc.dram_tensor("b", [O], F32, kind="ExternalInput")
    out_d = nc.dram_tensor("out_t", [O, T], F32, kind="ExternalOutput")

    with TileContext(nc) as tc, ExitStack() as ctx:
        ep = ctx.enter_context

        consts = ep(tc.tile_pool(name="consts", bufs=1))
        wpool = ep(tc.tile_pool(name="wpool", bufs=1))
        xpool = ep(tc.tile_pool(name="xpool", bufs=3))
        rpool = ep(tc.tile_pool(name="rpool", bufs=2))
        qpool = ep(tc.tile_pool(name="qpool", bufs=1))
        upool = ep(tc.tile_pool(name="upool", bufs=2))
        spool = ep(tc.tile_pool(name="spool", bufs=3))
        mm_ps = ep(tc.tile_pool(name="mm_ps", bufs=2, space="PSUM"))
        tr_ps = ep(tc.tile_pool(name="tr_ps", bufs=1, space="PSUM"))
        trb_ps = ep(tc.tile_pool(name="trb_ps", bufs=1, space="PSUM"))
        mi_ps = ep(tc.tile_pool(name="mi_ps", bufs=1, space="PSUM"))
        zz_ps = ep(tc.tile_pool(name="zz_ps", bufs=1, space="PSUM"))

        # ---------------- constants ----------------
        ident = consts.tile([128, 128], F32)
        make_identity(nc, ident[:])
        ident_bf = consts.tile([128, 128], BF16)
        make_identity(nc, ident_bf[:])
        magic = consts.tile([128, 1], F32)
        nc.vector.memset(magic[:], MAGIC)
        nmagic = consts.tile([128, 1], F32)
        nc.vector.memset(nmagic[:], -MAGIC)
        ones_col = consts.tile([128, 1], F32)
        nc.vector.memset(ones_col[:], 1.0)
        ones_row = consts.tile([1, 128], F32)
        nc.vector.memset(ones_row[:], 1.0)
        ones128 = consts.tile([128, 128], F32)
        nc.vector.memset(ones128[:], 1.0)
        zeros512 = consts.tile([128, 512], F32)
        nc.vector.memset(zeros512[:], 0.0)
        b_row = wpool.tile([1, O], F32, tag="brow")
        nc.sync.dma_start(out=b_row[:], in_=b_d[:].rearrange("(one o) -> one o", one=1))
        b_sb = consts.tile([128, NO], F32)
        for m in range(NO):
            bc = mi_ps.tile([128, 1], F32, tag="misc")
            nc.tensor.matmul(bc[:], lhsT=b_row[0:1, m * 128 : (m + 1) * 128],
                             rhs=ones_row[0:1, 0:1], start=True, stop=True)
            nc.vector.tensor_copy(out=b_sb[:, m : m + 1], in_=bc[:])

        # ---------------- weight phase ----------------
        # mean|W|: per-tile abs row-sums -> [128, NO] -> [128,1] -> PE
        # partition-reduce -> scalar.
        asum = consts.tile([128, NO], F32)
        for m in range(NO):
            w_t = wpool.tile([128, D], F32, tag="wload")
            nc.sync.dma_start(out=w_t[:], in_=w_d[m * 128 : (m + 1) * 128, :])
            nc.vector.tensor_reduce(
                out=asum[:, m : m + 1], in_=w_t[:], axis=mybir.AxisListType.X,
                op=Alu.add, apply_absolute_value=True)
        asum1 = consts.tile([128, 1], F32)
        nc.vector.tensor_reduce(
            out=asum1[:], in_=asum[:], axis=mybir.AxisListType.X, op=Alu.add)
        tot_ps = mi_ps.tile([1, 1], F32, tag="misc")
        nc.tensor.matmul(tot_ps[:], lhsT=asum1[:], rhs=ones_col[:],
                         start=True, stop=True)
        # ms[0,0] = mw = max(mean,1e-5);  ms[0,1] = sw = 1/mw
        ms = consts.tile([1, 2], F32)
        nc.vector.tensor_scalar(out=ms[:, 0:1], in0=tot_ps[:],
                                scalar1=1.0 / float(O * D), scalar2=1e-5,
                                op0=Alu.mult, op1=Alu.max)
        nc.vector.reciprocal(out=ms[:, 1:2], in_=ms[:, 0:1])
        bc_ps = mi_ps.tile([128, 2], F32, tag="misc")
        nc.tensor.matmul(bc_ps[:], lhsT=ones_row[:], rhs=ms[:],
                         start=True, stop=True)
        msb = consts.tile([128, 2], F32)
        nc.vector.tensor_copy(out=msb[:], in_=bc_ps[:])
        mean_b = msb[:, 0:1]  # mw broadcast over partitions
        sw_b = msb[:, 1:2]    # 1/mw broadcast

        # Quantize + transpose W -> wqt[dsub, m, k, osub] (bf16 ternary ints)
        wqt = qpool.tile([128, NO, NK, 128], BF16, tag="wqt")
        for m in range(NO):
            w_t = wpool.tile([128, D], F32, tag="wload2")
            nc.sync.dma_start(out=w_t[:], in_=w_d[m * 128 : (m + 1) * 128, :])
            rw = wpool.tile([128, D], F32, tag="rw")
            nc.scalar.activation(out=rw[:], in_=w_t[:], func=Act.Identity,
                                 bias=magic[:], scale=sw_b)
            rc = wpool.tile([128, D], F32, tag="rc")
            nc.vector.tensor_scalar(out=rc[:], in0=rw[:], scalar1=MAGIC,
                                    scalar2=1.0, op0=Alu.subtract, op1=Alu.min)
            wq = wpool.tile([128, D], BF16, tag="wq")
            nc.vector.tensor_scalar(out=wq[:], in0=rc[:], scalar1=-1.0,
                                    scalar2=None, op0=Alu.max)
            for half in range(NK // 4):
                tp = trb_ps.tile([128, 512], BF16, tag="trb")
                for j in range(4):
                    k = half * 4 + j
                    nc.tensor.transpose(
                        tp[:, j * 128 : (j + 1) * 128],
                        wq[:, k * 128 : (k + 1) * 128], ident_bf[:])
                nc.scalar.copy(out=wqt[:, m, half * 4 : half * 4 + 4, :],
                               in_=tp[:])

        # ---------------- streaming phase ----------------
        CH = min(1024, T)          # elementwise-unit width (t columns)
        NCH2 = T // CH             # number of units per o-tile
        NHT = CH // 512            # 512-col matmul groups per unit
        NJ = CH // 128             # 128-col t-tiles per unit

        zeros_ps = zz_ps.tile([128, 512], F32, tag="zz")
        nc.vector.memset(zeros_ps[:], 0.0)

        xqt_a = qpool.tile([128, max(NTT // 2, NJ), NK, 128], BF16, tag="xqt_a")
        xqt_b = qpool.tile([128, max(NTT // 2, NJ), NK, 128], BF16, tag="xqt_b")
        am127 = consts.tile([128, NTT], F32)  # amax'/127 per token
        rall = consts.tile([128, NTT], F32)   # 127/amax' per token
        carry = [consts.tile([128, 1], F32, tag=f"carry{o}", name=f"carry{o}")
                 for o in range(NO)]

        for c in range(NCH2):
            xqt = xqt_a if c % 2 == 0 else xqt_b
            cb = (c // 2) * NJ  # tile base within the ping-pong buffer
            # -- quantize + transpose the x-tiles of this chunk
            for tt in range(c * NJ, (c + 1) * NJ):
                x_t = xpool.tile([128, D], F32, tag="xload")
                nc.sync.dma_start(out=x_t[:], in_=x_d[tt * 128 : (tt + 1) * 128, :])
                amt = xpool.tile([128, 1], F32, tag="amt")
                nc.vector.tensor_reduce(
                    out=amt[:], in_=x_t[:], axis=mybir.AxisListType.X,
                    op=Alu.max, apply_absolute_value=True)
                nc.vector.tensor_scalar(
                    out=am127[:, tt : tt + 1], in0=amt[:], scalar1=1e-5,
                    scalar2=1.0 / 127.0, op0=Alu.max, op1=Alu.mult)
                nc.vector.reciprocal(out=rall[:, tt : tt + 1],
                                     in_=am127[:, tt : tt + 1])
                r_t = rpool.tile([128, D], F32, tag="r")
                nc.scalar.activation(out=r_t[:], in_=x_t[:], func=Act.Identity,
                                     bias=magic[:], scale=rall[:, tt : tt + 1])
                for half in range(NK // 4):
                    tp = tr_ps.tile([128, 512], F32, tag="tr")
                    for j in range(4):
                        k = half * 4 + j
                        nc.tensor.transpose(
                            tp[:, j * 128 : (j + 1) * 128],
                            r_t[:, k * 128 : (k + 1) * 128], ident[:])
                    # fused un-magic + bf16 cast on the PSUM->SBUF copy
                    nc.scalar.activation(
                        out=xqt[:, cb + tt - c * NJ, half * 4 : half * 4 + 4, :],
                        in_=tp[:], func=Act.Identity, bias=nmagic[:], scale=1.0)

            # -- per-token scale row via diag trick + PE broadcast
            diag = upool.tile([128, CH], F32, tag="diag")
            for j in range(NJ):
                nc.vector.tensor_scalar(
                    out=diag[:, j * 128 : (j + 1) * 128], in0=ident[:],
                    scalar1=am127[:, c * NJ + j : c * NJ + j + 1], scalar2=None,
                    op0=Alu.mult)
            s_raw = upool.tile([128, CH], F32, tag="sraw")
            for h in range(NHT):
                sbc = mi_ps.tile([128, 512], F32, tag="misc")
                nc.tensor.matmul(sbc[:], lhsT=ones128[:],
                                 rhs=diag[:, h * 512 : (h + 1) * 512],
                                 start=True, stop=True)
                nc.scalar.copy(out=s_raw[:, h * 512 : (h + 1) * 512], in_=sbc[:])

            # -- the NO output tiles of this chunk
            for o in range(NO):
                mm = mm_ps.tile([128, CH], F32, tag="mm")
                for h in range(NHT):
                    for k in range(NK):
                        nc.tensor.matmul(
                            mm[:, h * 512 : (h + 1) * 512],
                            lhsT=wqt[:, o, k, :],
                            rhs=xqt[:, cb + h * 4 : cb + h * 4 + 4, k, :],
                            start=(k == 0), stop=(k == NK - 1))
                m1 = upool.tile([128, CH], F32, tag="m1", bufs=3)
                nc.vector.tensor_tensor(out=m1[:], in0=mm[:], in1=s_raw[:],
                                        op=Alu.mult)
                nc.scalar.activation(out=m1[:], in_=m1[:], func=Act.Tanh,
                                     bias=b_sb[:, o : o + 1], scale=mean_b)
                st = spool.tile([128, CH], F32, tag="s")
                for h in range(NHT):
                    if c == 0 and h == 0:
                        init = 0.0
                    elif h == 0:
                        init = carry[o][:, 0:1]
                    else:
                        init = st[:, h * 512 - 1 : h * 512]
                    nc.vector.tensor_tensor_scan(
                        out=st[:, h * 512 : (h + 1) * 512],
                        data0=m1[:, h * 512 : (h + 1) * 512],
                        data1=zeros_ps[:], initial=init,
                        op0=Alu.add, op1=Alu.bypass)
                if c + 1 < NCH2:
                    nc.vector.tensor_copy(out=carry[o][:, 0:1],
                                          in_=st[:, CH - 1 : CH])
                u = upool.tile([128, CH], F32, tag="uf2", bufs=3)
                nc.scalar.activation(out=u[:], in_=st[:], func=Act.Identity,
                                     bias=magic[:], scale=g_coef)
                nc.vector.tensor_scalar(out=u[:], in0=u[:], scalar1=MAGIC,
                                        scalar2=TWO_PI / c_coef,
                                        op0=Alu.subtract, op1=Alu.mult)
                ot = upool.tile([128, CH], F32, tag="ot", bufs=3)
                nc.gpsimd.tensor_tensor(out=ot[:], in0=st[:], in1=u[:],
                                        op=Alu.subtract)
                nc.scalar.dma_start(
                    out=out_d[o * 128 : (o + 1) * 128, c * CH : (c + 1) * CH],
                    in_=ot[:])

    nc.finalize()
    return nc


def kernel(x: np.ndarray, W: np.ndarray, b: np.ndarray,
           cumsum_weight: np.ndarray) -> np.ndarray:
    B, T, D = x.shape
    O = W.shape[0]
    assert B == N_CORES
    cw = float(np.asarray(cumsum_weight).reshape(-1)[0])
    if cw == 0.0:
        # phase is identically 0; wrap(0) = 0
        return np.zeros((B, T, O), dtype=np.float32)
    nc = build(cw, T=T, D=D, O=O)
    x = np.ascontiguousarray(np.asarray(x, dtype=np.float32))
    W = np.ascontiguousarray(np.asarray(W, dtype=np.float32))
    b = np.ascontiguousarray(np.asarray(b, dtype=np.float32))
    in_maps = [{"x": x[i], "W": W, "b": b} for i in range(N_CORES)]
    res = run_bass_kernel_spmd(nc, in_maps, list(range(N_CORES)))
    c_coef = np.float32(PI * cw)
    # device returns phase/c in [O, T] layout; unshard = transpose + rescale
    return np.stack(
        [np.ascontiguousarray(res.results[i]["out_t"].T) * c_coef
         for i in range(N_CORES)], axis=0)
